# revision 4
# baseline (speedup 1.0000x reference)
"""Per-sample dynamic conv2d (VALID) on 8 Trainium2 NeuronCores — v3.

v3 = v2 (bf16 upload, DMA-xbar transpose loads, 9-tap PSUM accumulation)
with quad row-packing: each 2KB PSUM bank holds FOUR consecutive output rows
(4 x 128 f32).  The kernel taps are host-rearranged to [kw, j=2-kh] order so
that for input row r and kw, the taps of consecutive target rows are
CONTIGUOUS 128-col blocks in SBUF: one matmul with N=128*nrows covers a whole
run of rows inside a quad (psum cols (hp-4q)*128...).  This roughly halves
the matmul instruction count (same streamed columns) and evacuates four rows
per copy instead of one.

Quad q (rows 4q..4q+3) opens at (r=4q, kw=0) with start=True and closes at
(r=4q+5, kw=2) with stop=True (q=31 holds rows 124-125, closes at r=127).
All evacuations run on DVE (the PE's start-matmuls wait on them via bank
recycling, and the DVE queue never carries DMA-lane waits); stores batch 6
quads (24 rows) as bf16 on the ACT HWDGE ring (host casts the output back to
f32), keeping the SP ring free for the transpose loads.  ot bufs=6 decouples
evacuations from store completions (which queue behind transpose traffic on
the shared SDMA engines).
"""

import numpy as np
import ml_dtypes

import concourse.bass as bass
import concourse.mybir as mybir
from concourse.bass_utils import run_bass_kernel_spmd
from concourse.tile import TileContext

N_CORES = 8
B, H, W, C = 32, 128, 128, 128
KK = 3
BL = B // N_CORES            # samples per core
HO = WO = H - KK + 1         # 126
HW = H * W
XT_PAD = HW + 128            # matmuls read up to HW+2
NQ = (HO + 3) // 4           # 32 quads (last holds 2 rows)
QG = 6                       # quads per store group (24 rows; 6 stores/sample)

F32 = mybir.dt.float32
BF16 = mybir.dt.bfloat16


def _split_excess_waits(nc, limit=1):
    """walrus codegen rejects >1 sync-wait on several instruction kinds.
    Move excess waits onto preceding same-engine NoOps."""
    n = 0
    for bb in nc.m.functions[0].blocks:
        out = []
        changed = False
        for inst in bb.instructions:
            si = inst.sync_info
            if si is not None and len(si.on_wait) > limit:
                waits = list(si.on_wait)
                excess, keep = waits[:-limit], waits[-limit:]
                for i in range(0, len(excess), limit):
                    n += 1
                    out.append(
                        mybir.InstNoOp(
                            name=f"I-waitsplit-{n}",
                            engine=inst.engine,
                            bass_nofuse=True,
                            sync_info=mybir.SyncInfo(
                                on_wait=excess[i : i + limit], on_update=[]
                            ),
                        )
                    )
                inst.sync_info = mybir.SyncInfo(on_wait=keep, on_update=si.on_update)
                changed = True
            out.append(inst)
        if changed:
            bb.instructions = out
    return n


def _build():
    nc = bass.Bass()
    Xd = nc.declare_dram_parameter("X", [BL, HW, C], BF16, isOutput=False)
    # host-rearranged: t = kw*3 + j with j = 2-kh
    Kd = nc.declare_dram_parameter("kern", [BL, KK * KK, C, C], BF16, isOutput=False)
    Od = nc.declare_dram_parameter("out", [BL, HO, WO, C], BF16, isOutput=True)

    with TileContext(nc) as tc:
        with (
            # outb/kall first: the framework preamble's SBUF staging region
            # overlaps the lowest pool addresses, and its WAR waits must land
            # on these late-written buffers, not on the startup-critical
            # transpose target xt.
            tc.tile_pool(name="outb", bufs=6) as p_out,
            tc.tile_pool(name="kt", bufs=3) as p_k,
            tc.tile_pool(name="xt", bufs=3) as p_xt,
            tc.tile_pool(name="pacc", bufs=8, space="PSUM") as p_acc,
        ):
            def emit_load(b, nchunks):
                xt = p_xt.tile([C, XT_PAD], BF16, tag="xt")
                step = HW // nchunks
                for c0 in range(0, HW, step):
                    nc.sync.dma_start(
                        out=xt[:, c0 : c0 + step],
                        in_=Xd[b, c0 : c0 + step, :],
                        transpose=True,
                    )
                # small; SWDGE ring is idle — keeps it off the transpose FIFO
                kall = p_k.tile([C, KK * KK * C], BF16, tag="kall")
                nc.gpsimd.dma_start(
                    out=kall[:, :].rearrange("ci (t co) -> ci t co", t=KK * KK),
                    in_=Kd[b].rearrange("t ci co -> ci t co"),
                )
                return {"kall": kall, "xt": xt}

            def emit_compute(b, st):
                kall, xt = st["kall"], st["xt"]
                live = {}
                ot = None
                for r in range(H):
                    for kw in range(KK):
                        x_sl = xt[:, r * 128 + kw : r * 128 + kw + 128]
                        a, hp_hi = max(0, r - 2), min(r, HO - 1)
                        while a <= hp_hi:
                            q = a // 4
                            b_end = min(hp_hi, 4 * q + 3)
                            nt = b_end - a + 1
                            if q not in live:
                                pr = p_acc.tile([W, 512], F32, tag="P")
                                live[q] = pr
                            j_a = a - r + 2
                            close_r = 4 * q + 5 if q < NQ - 1 else H - 1
                            nc.tensor.matmul(
                                live[q][:, (a - 4 * q) * C : (a - 4 * q + nt) * C],
                                x_sl,
                                kall[:, (kw * KK + j_a) * C : (kw * KK + j_a + nt) * C],
                                start=(r == 4 * q and kw == 0),
                                stop=(r == close_r and kw == KK - 1),
                            )
                            a = b_end + 1
                    # quads closing at this r
                    closed = []
                    if r >= 5 and (r - 5) % 4 == 0:
                        closed.append((r - 5) // 4)
                    if r == H - 1:
                        closed.append(NQ - 1)
                    for q in closed:
                        nrows = min(4, HO - 4 * q)
                        g = q // QG          # store group
                        k = q % QG           # quad slot within group
                        if k == 0:
                            ot = p_out.tile([WO, QG * 4 * C], BF16, tag="ot")
                        src = live.pop(q)[0:WO, 0 : nrows * C]
                        dst = ot[0:WO, k * 4 * C : (k * 4 + nrows) * C]
                        # ALL evacs on DVE: the PE's start-matmuls wait on
                        # these via bank recycling, and the DVE queue never
                        # carries DMA-lane waits (unlike ACT, whose store
                        # DMAs wait on lanes recycled from the transposes).
                        nc.vector.tensor_copy(dst, src)
                        last_in_group = (k == QG - 1) or (q == NQ - 1)
                        if last_in_group:
                            base = g * QG * 4
                            nr = k * 4 + nrows
                            nc.scalar.dma_start(
                                out=Od[b, base : base + nr].rearrange(
                                    "h w c -> w h c"
                                ),
                                in_=ot[0:WO, 0 : nr * C].rearrange(
                                    "w (h c) -> w h c", h=nr
                                ),
                            )

            # 2 loads + 6 stores per sample = 8 HWDGE DMAs, matching the 8
            # DMAHW completion-sem lanes Tile round-robins: each DMA's
            # lane-recycling wait lands on the same DMA kind one sample
            # back, so loads never gate on stores (which would make them
            # just-in-time instead of prefetched).
            st = emit_load(0, nchunks=8)
            for b in range(BL):
                nxt = emit_load(b + 1, nchunks=2) if b + 1 < BL else None
                emit_compute(b, st)
                st = nxt

    _split_excess_waits(nc)
    return nc


_CACHE = {}


def _get_nc():
    if "nc" not in _CACHE:
        _CACHE["nc"] = _build()
    return _CACHE["nc"]


def _run(X, kern, **kw):
    Xb = X.astype(ml_dtypes.bfloat16).reshape(B, HW, C)
    # [B, kh, kw, ci, co] -> [B, kw, j=2-kh, ci, co] -> [B, 9, ci, co]
    Kb = (
        kern.astype(ml_dtypes.bfloat16)[:, ::-1]
        .transpose(0, 2, 1, 3, 4)
        .reshape(B, KK * KK, C, C)
    )
    in_maps = [
        {
            "X": np.ascontiguousarray(Xb[c * BL : (c + 1) * BL]),
            "kern": np.ascontiguousarray(Kb[c * BL : (c + 1) * BL]),
        }
        for c in range(N_CORES)
    ]
    last_err = None
    for _attempt in range(3):
        try:
            res = run_bass_kernel_spmd(
                _get_nc(), in_maps, list(range(N_CORES)), **kw
            )
            break
        except Exception as e:  # transient NRT_EXEC_UNIT_UNRECOVERABLE etc.
            last_err = e
    else:
        raise last_err
    out = np.concatenate(
        [np.asarray(res.results[c]["out"]).astype(np.float32) for c in range(N_CORES)],
        axis=0,
    )
    return out, res


def kernel(X, kernel):
    X = np.ascontiguousarray(X, dtype=np.float32)
    kern = np.ascontiguousarray(kernel, dtype=np.float32)
    out, _ = _run(X, kern)
    return out


# revision 5
# speedup vs baseline: 1.1649x; 1.1649x over previous
"""Per-sample dynamic conv2d (VALID) on 8 Trainium2 NeuronCores — v3.

v3 = v2 (bf16 upload, DMA-xbar transpose loads, 9-tap PSUM accumulation)
with quad row-packing: each 2KB PSUM bank holds FOUR consecutive output rows
(4 x 128 f32).  The kernel taps are host-rearranged to [kw, j=2-kh] order so
that for input row r and kw, the taps of consecutive target rows are
CONTIGUOUS 128-col blocks in SBUF: one matmul with N=128*nrows covers a whole
run of rows inside a quad (psum cols (hp-4q)*128...).  This roughly halves
the matmul instruction count (same streamed columns) and evacuates four rows
per copy instead of one.

Quad q (rows 4q..4q+3) opens at (r=4q, kw=0) with start=True and closes at
(r=4q+5, kw=2) with stop=True (q=31 holds rows 124-125, closes at r=127).
All evacuations run on DVE (the PE's start-matmuls wait on them via bank
recycling, and the DVE queue never carries DMA-lane waits); stores batch 6
quads (24 rows) as bf16 on the ACT HWDGE ring (host casts the output back to
f32), keeping the SP ring free for the transpose loads.  ot bufs=6 decouples
evacuations from store completions (which queue behind transpose traffic on
the shared SDMA engines).
"""

import numpy as np
import ml_dtypes

import concourse.bass as bass
import concourse.mybir as mybir
from concourse.bass_utils import run_bass_kernel_spmd
from concourse.tile import TileContext

N_CORES = 8
B, H, W, C = 32, 128, 128, 128
KK = 3
BL = B // N_CORES            # samples per core
HO = WO = H - KK + 1         # 126
HW = H * W
XT_PAD = HW + 128            # matmuls read up to HW+2
NQ = (HO + 3) // 4           # 32 quads (last holds 2 rows)
QG = 6                       # quads per store group (24 rows; 6 stores/sample)

F32 = mybir.dt.float32
BF16 = mybir.dt.bfloat16


def _split_excess_waits(nc, limit=1):
    """walrus codegen rejects >1 sync-wait on several instruction kinds.
    Move excess waits onto preceding same-engine NoOps."""
    n = 0
    for bb in nc.m.functions[0].blocks:
        out = []
        changed = False
        for inst in bb.instructions:
            si = inst.sync_info
            if si is not None and len(si.on_wait) > limit:
                waits = list(si.on_wait)
                excess, keep = waits[:-limit], waits[-limit:]
                for i in range(0, len(excess), limit):
                    n += 1
                    out.append(
                        mybir.InstNoOp(
                            name=f"I-waitsplit-{n}",
                            engine=inst.engine,
                            bass_nofuse=True,
                            sync_info=mybir.SyncInfo(
                                on_wait=excess[i : i + limit], on_update=[]
                            ),
                        )
                    )
                inst.sync_info = mybir.SyncInfo(on_wait=keep, on_update=si.on_update)
                changed = True
            out.append(inst)
        if changed:
            bb.instructions = out
    return n


def _build():
    nc = bass.Bass()
    Xd = nc.declare_dram_parameter("X", [BL, HW, C], BF16, isOutput=False)
    # host-rearranged: t = kw*3 + j with j = 2-kh
    Kd = nc.declare_dram_parameter("kern", [BL, KK * KK, C, C], BF16, isOutput=False)
    Od = nc.declare_dram_parameter("out", [BL, HO, WO, C], BF16, isOutput=True)

    with TileContext(nc) as tc:
        with (
            tc.tile_pool(name="xt", bufs=3) as p_xt,
            tc.tile_pool(name="kt", bufs=3) as p_k,
            tc.tile_pool(name="outb", bufs=6) as p_out,
            tc.tile_pool(name="pacc", bufs=8, space="PSUM") as p_acc,
        ):
            def emit_load(b, nchunks):
                xt = p_xt.tile([C, XT_PAD], BF16, tag="xt")
                step = HW // nchunks
                for c0 in range(0, HW, step):
                    nc.sync.dma_start(
                        out=xt[:, c0 : c0 + step],
                        in_=Xd[b, c0 : c0 + step, :],
                        transpose=True,
                    )
                # small; SWDGE ring is idle — keeps it off the transpose FIFO
                kall = p_k.tile([C, KK * KK * C], BF16, tag="kall")
                nc.gpsimd.dma_start(
                    out=kall[:, :].rearrange("ci (t co) -> ci t co", t=KK * KK),
                    in_=Kd[b].rearrange("t ci co -> ci t co"),
                )
                return {"kall": kall, "xt": xt}

            def emit_compute(b, st):
                kall, xt = st["kall"], st["xt"]
                live = {}
                ot = None
                for r in range(H):
                    for kw in range(KK):
                        x_sl = xt[:, r * 128 + kw : r * 128 + kw + 128]
                        a, hp_hi = max(0, r - 2), min(r, HO - 1)
                        while a <= hp_hi:
                            q = a // 4
                            b_end = min(hp_hi, 4 * q + 3)
                            nt = b_end - a + 1
                            if q not in live:
                                pr = p_acc.tile([W, 512], F32, tag="P")
                                live[q] = pr
                            j_a = a - r + 2
                            close_r = 4 * q + 5 if q < NQ - 1 else H - 1
                            nc.tensor.matmul(
                                live[q][:, (a - 4 * q) * C : (a - 4 * q + nt) * C],
                                x_sl,
                                kall[:, (kw * KK + j_a) * C : (kw * KK + j_a + nt) * C],
                                start=(r == 4 * q and kw == 0),
                                stop=(r == close_r and kw == KK - 1),
                            )
                            a = b_end + 1
                    # quads closing at this r
                    closed = []
                    if r >= 5 and (r - 5) % 4 == 0:
                        closed.append((r - 5) // 4)
                    if r == H - 1:
                        closed.append(NQ - 1)
                    for q in closed:
                        nrows = min(4, HO - 4 * q)
                        g = q // QG          # store group
                        k = q % QG           # quad slot within group
                        if k == 0:
                            ot = p_out.tile([WO, QG * 4 * C], BF16, tag="ot")
                        src = live.pop(q)[0:WO, 0 : nrows * C]
                        dst = ot[0:WO, k * 4 * C : (k * 4 + nrows) * C]
                        # ALL evacs on DVE: the PE's start-matmuls wait on
                        # these via bank recycling, and the DVE queue never
                        # carries DMA-lane waits (unlike ACT, whose store
                        # DMAs wait on lanes recycled from the transposes).
                        nc.vector.tensor_copy(dst, src)
                        last_in_group = (k == QG - 1) or (q == NQ - 1)
                        if last_in_group:
                            base = g * QG * 4
                            nr = k * 4 + nrows
                            nc.scalar.dma_start(
                                out=Od[b, base : base + nr].rearrange(
                                    "h w c -> w h c"
                                ),
                                in_=ot[0:WO, 0 : nr * C].rearrange(
                                    "w (h c) -> w h c", h=nr
                                ),
                            )

            # 2 loads + 6 stores per sample = 8 HWDGE DMAs, matching the 8
            # DMAHW completion-sem lanes Tile round-robins: each DMA's
            # lane-recycling wait lands on the same DMA kind one sample
            # back, so loads never gate on stores (which would make them
            # just-in-time instead of prefetched).
            st = emit_load(0, nchunks=8)
            for b in range(BL):
                nxt = emit_load(b + 1, nchunks=2) if b + 1 < BL else None
                emit_compute(b, st)
                st = nxt

    _split_excess_waits(nc)
    return nc


_CACHE = {}


def _get_nc():
    if "nc" not in _CACHE:
        _CACHE["nc"] = _build()
    return _CACHE["nc"]


def _run(X, kern, **kw):
    Xb = X.astype(ml_dtypes.bfloat16).reshape(B, HW, C)
    # [B, kh, kw, ci, co] -> [B, kw, j=2-kh, ci, co] -> [B, 9, ci, co]
    Kb = (
        kern.astype(ml_dtypes.bfloat16)[:, ::-1]
        .transpose(0, 2, 1, 3, 4)
        .reshape(B, KK * KK, C, C)
    )
    in_maps = [
        {
            "X": np.ascontiguousarray(Xb[c * BL : (c + 1) * BL]),
            "kern": np.ascontiguousarray(Kb[c * BL : (c + 1) * BL]),
        }
        for c in range(N_CORES)
    ]
    last_err = None
    for _attempt in range(3):
        try:
            res = run_bass_kernel_spmd(
                _get_nc(), in_maps, list(range(N_CORES)), **kw
            )
            break
        except Exception as e:  # transient NRT_EXEC_UNIT_UNRECOVERABLE etc.
            last_err = e
    else:
        raise last_err
    out = np.concatenate(
        [np.asarray(res.results[c]["out"]).astype(np.float32) for c in range(N_CORES)],
        axis=0,
    )
    return out, res


def kernel(X, kernel):
    X = np.ascontiguousarray(X, dtype=np.float32)
    kern = np.ascontiguousarray(kernel, dtype=np.float32)
    out, _ = _run(X, kern)
    return out
